# revision 31
# baseline (speedup 1.0000x reference)
"""Locally-connected convolution (unshared weights) on 8 Trainium2 NeuronCores.

out[b,o,i,j] = sum_{c,u,v} x[b,c,i+u,j+v] * weight[i,j,o,c,u,v]
  x: [64, 64, 32, 32] f32, weight: [28, 28, 128, 64, 5, 5] f32 -> out [64, 128, 28, 28]

Strategy (v2, "pairs" layout): each of the 784 output positions is an
independent GEMM [K=1600] x [K=1600, O=128] over B=64 batch vectors.  Shard
the 784 positions across 8 cores (98 each, raster-contiguous).

Weights are cast host-side to float8 E3M4 (x32 scale; x is pre-scaled by
1/32 in fp16 so no on-device rescale is needed).  This halves the dominant
HBM traffic (weights are used exactly once).  The matmul is "flipped"
vs. v1: weights are the stationary lhsT [K, O=128] and x is the moving rhs
[K, B=64], so each matmul streams only 64 rows and fills all 128 PSUM
partitions -> 2x fewer PE cycles.

K decomposition per position (1600 = 25 taps x 64 ch):
  x is stored ONCE per core in "column pair" layout: partitions 0-63 hold
  channels of EVEN input columns, partitions 64-127 channels of ODD columns;
  free axis is (row h, column-pair cp, batch b).  A [128, 64] slice at
  (h, cp) yields two adjacent-column taps at K=128.  Positions alternate
  column parity, so every position gets 10 such pair chunks (u x 2) plus 5
  leftover single taps (K=64, lower partitions for even positions / upper
  for odd).  Leftover-tap weights of an (even, odd) position pair share one
  [128, 128] block -> zero padding in the weight stream (20.07 MB/core).
"""

import numpy as np

B, C, H, W = 64, 64, 32, 32
ROWS = COLS = 28
O, KH, KW = 128, 5, 5
NCORES = 8
PPC = (ROWS * COLS) // NCORES  # 98 positions per core
NPAIR = PPC // 2               # 49 (even, odd) position pairs
XROWS, XW = 8, 36              # sheared x grid: 8 input rows x 36 cols
PAIRS = XW // 2                # 18 column pairs per sheared row
PAIRB = 2 * 10 * O + 5 * O     # weight bytes per partition per pair = 3200
WSCALE = 32.0                  # weights x32 into E3M4 range; x carries /32
OBLK = 7                       # positions per output block/DMA
WLA = 16                       # wtile pool depth (pairs in flight)
XHALF = 9 * B                  # x row loaded in two 9-pair halves
XRROWS = 6                     # row-pair workspace rows (di+u0 <= 5)


def _core_geom(k):
    p0 = PPC * k
    return p0 // COLS, p0 % COLS  # r0 (first input/output row), s0 in {0, 14}


def _pos_slot(t):
    """Relative position t in [0,98) -> (di, w2) grid coords shared by all cores."""
    di, jj = t // COLS, t % COLS
    return di, jj + (4 if jj >= 14 else 0)


def _build_xs(x_chwb, k):
    """x_chwb: [C,H,W,B] f32 -> sheared per-core grid [C, XROWS, XW, B]."""
    r0, s0 = _core_geom(k)
    xs = np.zeros((C, XROWS, XW, B), dtype=x_chwb.dtype)
    for h in range(XROWS):
        if s0 == 0:
            xs[:, h, 0:18] = x_chwb[:, r0 + h, 0:18]
            xs[:, h, 18:36] = x_chwb[:, r0 + h, 14:32]
        else:
            xs[:, h, 0:18] = x_chwb[:, r0 + h, 14:32]
            if r0 + h + 1 < H:
                xs[:, h, 18:36] = x_chwb[:, r0 + h + 1, 0:18]
    return xs


def _build_xp(x_chwb, k):
    """-> [XROWS, 128, PAIRS*B] f16, partition g*64+c = channel c of col 2cp+g."""
    xs = _build_xs(x_chwb, k) * np.float32(1.0 / WSCALE)
    # [C, XROWS, PAIRS, 2, B] -> [2, C, XROWS, PAIRS, B]
    xg = xs.reshape(C, XROWS, PAIRS, 2, B).transpose(3, 0, 1, 2, 4)
    xp = xg.reshape(128, XROWS, PAIRS * B).transpose(1, 0, 2)
    return np.ascontiguousarray(xp).astype(np.float16)


def _build_xr(x_chwb, k):
    """Row-pair workspace [XRROWS, 128, XW*B] f16: partition g*64+c of row
    slot h = channel c of sheared row h+g (full 36-column resolution)."""
    xs = _build_xs(x_chwb, k) * np.float32(1.0 / WSCALE)
    xr = np.zeros((XRROWS, 2, C, XW * B), np.float32)
    for h in range(XRROWS):
        xr[h, 0] = xs[:, h].reshape(C, XW * B)
        xr[h, 1] = xs[:, h + 1].reshape(C, XW * B)
    return np.ascontiguousarray(xr.reshape(XRROWS, 128, XW * B)).astype(np.float16)


def _abs_pos(k, t):
    p = PPC * k + t
    return p // COLS, p % COLS


def _build_wt(weight, k):
    """weight [ROWS,COLS,O,C,KH,KW] f32 -> per-core [128, NPAIR*PAIRB] E3M4.

    Per pair p (positions te=2p, to=2p+1), per-partition byte layout:
      [0,1280):    te pair chunks, (u,q)-major, each [128, O]; partition
                   g*64+c holds w[o, c, u, v = 2q + g + par(t)]
      [1280,1920): leftover-tap blocks, each [128, O]:
        +0, +O:    te row-pair chunks (u0 in {0,2}): partition g*64+c =
                   w_te[o, c, u0+g, 4]   (contract against XR)
        +2O, +3O:  to row-pair chunks: w_to[o, c, u0+g, 0]
        +4O:       shared u=4 block: partitions 0:64 = w_te[o,c,4,4],
                   64:128 = w_to[o,c,4,0]  (contract against XT)
      [1920,3200): to pair chunks
    (te's operands live in [0,1920) so the leading DMA of pair 0 can be
    split there for a faster pipeline start.)
    """
    import ml_dtypes

    ii, jj = zip(*[_abs_pos(k, t) for t in range(PPC)])
    wc = weight[list(ii), list(jj)]  # [PPC, O, C, KH, KW] f32
    # t parity == jj parity == w2 parity for every core (offsets are even)
    WT = np.zeros((2, C, NPAIR, PAIRB), np.float32)  # [g, c, p, col]
    for u in range(KH):
        for q in range(2):
            for g in range(2):
                for half in range(2):
                    v = 2 * q + g + half
                    blk = wc[half::2, :, :, u, v]  # [NPAIR, O, C]
                    col = half * 1920 + (2 * u + q) * O
                    WT[g, :, :, col:col + O] = blk.transpose(2, 0, 1)
    for g in range(2):
        for i, u0 in enumerate((0, 2)):
            WT[g, :, :, 1280 + i * O:1280 + (i + 1) * O] = \
                wc[0::2, :, :, u0 + g, 4].transpose(2, 0, 1)
            WT[g, :, :, 1280 + (2 + i) * O:1280 + (3 + i) * O] = \
                wc[1::2, :, :, u0 + g, 0].transpose(2, 0, 1)
    WT[0, :, :, 1280 + 4 * O:1280 + 5 * O] = wc[0::2, :, :, 4, 4].transpose(2, 0, 1)
    WT[1, :, :, 1280 + 4 * O:1280 + 5 * O] = wc[1::2, :, :, 4, 0].transpose(2, 0, 1)
    wt = np.ascontiguousarray(WT.reshape(128, NPAIR * PAIRB) * np.float32(WSCALE))
    return wt.astype(ml_dtypes.float8_e3m4)


def _chunks(t):
    """Position t -> (di, 10 pair descriptors, 2 row-pair + 1 u4 descriptor).

    Descriptors are (kind, u, cp, woff, g): kind "pair" contracts XT with
    K=128 (two adjacent columns), "lp" contracts XR with K=128 (two
    adjacent rows of the leftover column), "lone" is the final K=64 u=4
    tap via XT.  The device emits all K=128 chunks of an output block
    first, then the K=64 chunks grouped by parity, so 128<->64 PE
    tile-config switches happen per block, not per position.
    """
    di, w2 = _pos_slot(t)
    par = w2 % 2
    half = t % 2
    cp0 = (w2 + par) // 2
    col_l = w2 + 4 if par == 0 else w2      # leftover-tap column
    cpl = col_l // 2
    pairs = [("pair", u, cp0 + q, half * 1920 + (2 * u + q) * O, 0)
             for u in range(KH) for q in range(2)]
    lps = [("lp", u0, col_l, 1280 + (2 * half + i) * O, 0)
           for i, u0 in enumerate((0, 2))]
    lone = ("lone", 4, cpl, 1280 + 4 * O, par)
    return di, pairs, lps, lone


def _emulate_core(xp, wt, xr):
    """Pure-numpy emulation of the device program (mirrors AP arithmetic)."""
    xpf = xp.astype(np.float32)                      # [8, 128, PAIRS*B]
    xrf = xr.astype(np.float32)                      # [6, 128, XW*B]
    wtf = wt.astype(np.float32).reshape(128, NPAIR, PAIRB)
    out = np.zeros((128, PPC, B), np.float32)
    for t in range(PPC):
        p = t // 2
        di, pairs, lps, lone = _chunks(t)
        acc = np.zeros((128, B), np.float32)
        for kind, u, cp, woff, g in pairs + lps + [lone]:
            if kind == "pair":
                lhsT = wtf[:, p, woff:woff + O]          # [128, 128]
                rhs = xpf[di + u, :, cp * B:(cp + 1) * B]  # [128, 64]
            elif kind == "lp":
                lhsT = wtf[:, p, woff:woff + O]
                rhs = xrf[di + u, :, cp * B:(cp + 1) * B]  # cp = column here
            else:
                lhsT = wtf[g * 64:(g + 1) * 64, p, woff:woff + O]
                rhs = xpf[di + u, g * 64:(g + 1) * 64, cp * B:(cp + 1) * B]
            acc += lhsT.T @ rhs
        out[:, t, :] = acc
    return out  # [O, PPC, B]; scale already folded via x/32 * w*32


def _assemble(outs):
    """list of 8 per-core [128, PPC*B] -> [B, O, ROWS, COLS] f32."""
    full = np.concatenate(
        [np.asarray(o, np.float32).reshape(O, PPC, B) for o in outs], axis=1)
    return np.ascontiguousarray(full.transpose(2, 0, 1)).reshape(B, O, ROWS, COLS)


_PROG_CACHE = {}


def _build_program():
    if "nc" in _PROG_CACHE:
        return _PROG_CACHE["nc"]
    import concourse.bass as bass
    import concourse.tile as tile
    from concourse import bacc, mybir

    f8, f16, f32 = mybir.dt.float8e3, mybir.dt.float16, mybir.dt.float32
    nc = bacc.Bacc("TRN2", target_bir_lowering=False, debug=False, num_devices=NCORES)
    xp_d = nc.dram_tensor("xp", [XROWS, 128, PAIRS * B], f16, kind="ExternalInput")
    xr_d = nc.dram_tensor("xr", [XRROWS, 128, XW * B], f16, kind="ExternalInput")
    wt_d = nc.dram_tensor("wt", [128, NPAIR * PAIRB], f8, kind="ExternalInput")
    out_d = nc.dram_tensor("out", [128, PPC * B], f16, kind="ExternalOutput")

    with tile.TileContext(nc) as tc:
        with tc.tile_pool(name="xpool", bufs=1) as xpool, \
             tc.tile_pool(name="wpool", bufs=WLA) as wpool, \
             tc.tile_pool(name="opool", bufs=3) as opool, \
             tc.tile_pool(name="psum", bufs=8, space="PSUM") as ppool:
            xp, xr, wt, outp = xp_d.ap(), xr_d.ap(), wt_d.ap(), out_d.ap()
            XT = [xpool.tile([128, PAIRS * B], f16, name=f"x{h}", tag=f"x{h}")
                  for h in range(XROWS)]
            XR = [xpool.tile([128, XW * B], f16, name=f"xr{h}", tag=f"xr{h}")
                  for h in range(XRROWS)]
            # Weight DMAs carry GROUPS[i] pairs each: big transfers amortize
            # the per-DMA SEQ/DGE overhead (~1.2us) so two HWDGE queues keep
            # the 16 DMA engines saturated; the leading groups stay small
            # (and pair 0 is split) for a fast pipeline start.
            GROUPS = [1, 1] + [2] * 23 + [1]
            g0 = [0]
            for n in GROUPS:
                g0.append(g0[-1] + n)
            pair_loc = {}
            for gi, n in enumerate(GROUPS):
                for l in range(n):
                    pair_loc[g0[gi] + l] = (gi, l * PAIRB)
            wtiles = [wpool.tile([128, n * PAIRB], f8, name=f"w{gi}", tag="wt")
                      for gi, n in enumerate(GROUPS)]
            weng = [nc.scalar, nc.sync]

            def load_x(h, hf, eng):
                eng.dma_start(XT[h][:, hf * XHALF:(hf + 1) * XHALF],
                              xp[h, :, hf * XHALF:(hf + 1) * XHALF])

            def load_w(gi):
                c0, c1 = g0[gi] * PAIRB, g0[gi + 1] * PAIRB
                weng[gi % 2].dma_start(wtiles[gi][:], wt[:, c0:c1])

            def load_xr(h, eng):
                eng.dma_start(XR[h][:], xr[h])

            # Emission order == per-queue FIFO order.  Position 0 needs x
            # rows 0-4 (first halves) + pair 0's [0,1920) slice; XR rows 0,2
            # at the end of block 0; x rows 5-7 only matter from t=28,
            # second halves (xb) from t=14.
            nc.sync.dma_start(wtiles[0][:, 0:1920], wt[:, 0:1920])
            for h in range(3):
                load_x(h, 0, nc.scalar)
            nc.sync.dma_start(wtiles[0][:, 1920:PAIRB], wt[:, 1920:PAIRB])
            load_x(3, 0, nc.scalar)
            load_x(4, 0, nc.scalar)
            load_w(1)   # sync
            load_w(2)   # scalar
            load_xr(0, nc.sync)
            load_xr(2, nc.scalar)
            load_w(3)   # sync
            for h in range(5, XROWS):
                load_x(h, 0, nc.scalar)
            load_w(4)
            load_xr(1, nc.sync)
            load_xr(3, nc.scalar)
            for h in range(0, XROWS, 2):
                load_x(h, 1, nc.sync)
            for h in range(1, XROWS, 2):
                load_x(h, 1, nc.scalar)
            load_xr(4, nc.sync)
            load_xr(5, nc.scalar)
            for gi in range(5, len(GROUPS)):
                load_w(gi)  # flow-controlled by wpool depth

            def mm(ps, p, di, ch, start, stop):
                kind, u, cp, woff, g = ch
                gi, poff = pair_loc[p]
                if kind == "pair":
                    lhsT = wtiles[gi][:, poff + woff:poff + woff + O]
                    rhs = XT[di + u][:, cp * B:(cp + 1) * B]
                elif kind == "lp":
                    lhsT = wtiles[gi][:, poff + woff:poff + woff + O]
                    rhs = XR[di + u][:, cp * B:(cp + 1) * B]
                else:
                    lhsT = wtiles[gi][g * 64:g * 64 + 64, poff + woff:poff + woff + O]
                    rhs = XT[di + u][g * 64:g * 64 + 64, cp * B:(cp + 1) * B]
                nc.tensor.matmul(ps[:], lhsT, rhs, start=start, stop=stop)

            for t0 in range(0, PPC, OBLK):
                otile = opool.tile([128, OBLK * B], f16, tag="ot")
                pss, parts = {}, {}
                for t in range(t0, t0 + OBLK):
                    di, pairs, lps, lone = _chunks(t)
                    ps = ppool.tile([128, B], f32, tag="ps")
                    pss[t], parts[t] = ps, (di, lone)
                    for i, ch in enumerate(pairs + lps):
                        mm(ps, t // 2, di, ch, start=(i == 0), stop=False)
                for par in (t0 % 2, 1 - t0 % 2):
                    for t in range(t0, t0 + OBLK):
                        if t % 2 != par:
                            continue
                        di, lone = parts[t]
                        mm(pss[t], t // 2, di, lone, start=False, stop=True)
                        nc.vector.tensor_copy(
                            otile[:, (t - t0) * B:(t - t0 + 1) * B], pss[t][:])
                oeng = nc.sync if t0 + OBLK >= PPC else nc.gpsimd
                oeng.dma_start(outp[:, t0 * B:(t0 + OBLK) * B], otile[:])

    nc.compile()
    _PROG_CACHE["nc"] = nc
    return nc


def _make_in_maps(x, weight):
    x_chwb = np.ascontiguousarray(
        np.asarray(x, np.float32).transpose(1, 2, 3, 0))
    w32 = np.asarray(weight, np.float32)
    return [{"xp": _build_xp(x_chwb, k), "wt": _build_wt(w32, k),
             "xr": _build_xr(x_chwb, k)}
            for k in range(NCORES)]


def kernel(x, weight):
    from concourse.bass_utils import run_bass_kernel_spmd

    nc = _build_program()
    in_maps = _make_in_maps(x, weight)
    res = run_bass_kernel_spmd(nc, in_maps, core_ids=list(range(NCORES)))
    return _assemble([res.results[k]["out"] for k in range(NCORES)])


# revision 45
# speedup vs baseline: 1.1594x; 1.1594x over previous
"""Locally-connected convolution (unshared weights) on 8 Trainium2 NeuronCores.

out[b,o,i,j] = sum_{c,u,v} x[b,c,i+u,j+v] * weight[i,j,o,c,u,v]
  x: [64, 64, 32, 32] f32, weight: [28, 28, 128, 64, 5, 5] f32 -> out [64, 128, 28, 28]

Strategy (v2, "pairs" layout): each of the 784 output positions is an
independent GEMM [K=1600] x [K=1600, O=128] over B=64 batch vectors.  Shard
the 784 positions across 8 cores (98 each, raster-contiguous).

Weights are cast host-side to float8 E3M4 (x32 scale; x is pre-scaled by
1/32 in fp16 so no on-device rescale is needed).  This halves the dominant
HBM traffic (weights are used exactly once).  The matmul is "flipped"
vs. v1: weights are the stationary lhsT [K, O=128] and x is the moving rhs
[K, B=64], so each matmul streams only 64 rows and fills all 128 PSUM
partitions -> 2x fewer PE cycles.

K decomposition per position (1600 = 25 taps x 64 ch):
  x is stored ONCE per core in "column pair" layout: partitions 0-63 hold
  channels of EVEN input columns, partitions 64-127 channels of ODD columns;
  free axis is (row h, column-pair cp, batch b).  A [128, 64] slice at
  (h, cp) yields two adjacent-column taps at K=128.  Positions alternate
  column parity, so every position gets 10 such pair chunks (u x 2) plus 5
  leftover single taps (K=64, lower partitions for even positions / upper
  for odd).  Leftover-tap weights of an (even, odd) position pair share one
  [128, 128] block -> zero padding in the weight stream (20.07 MB/core).
"""

import numpy as np

B, C, H, W = 64, 64, 32, 32
ROWS = COLS = 28
O, KH, KW = 128, 5, 5
NCORES = 8
PPC = (ROWS * COLS) // NCORES  # 98 positions per core
NPAIR = PPC // 2               # 49 (even, odd) position pairs
XROWS, XW = 8, 36              # sheared x grid: 8 input rows x 36 cols
PAIRS = XW // 2                # 18 column pairs per sheared row
PAIRB = 2 * 10 * O + 5 * O     # weight bytes per partition per pair = 3200
WSCALE = 32.0                  # weights x32 into E3M4 range; x carries /32
OBLK = 7                       # positions per output block/DMA
WLA = 16                       # wtile pool depth (pairs in flight)
XHALF = 9 * B                  # x row loaded in two 9-pair halves
XRROWS = 6                     # row-pair workspace rows (di+u0 <= 5)


def _core_geom(k):
    p0 = PPC * k
    return p0 // COLS, p0 % COLS  # r0 (first input/output row), s0 in {0, 14}


def _pos_slot(t):
    """Relative position t in [0,98) -> (di, w2) grid coords shared by all cores."""
    di, jj = t // COLS, t % COLS
    return di, jj + (4 if jj >= 14 else 0)


def _build_xs(x_chwb, k):
    """x_chwb: [C,H,W,B] f32 -> sheared per-core grid [C, XROWS, XW, B]."""
    r0, s0 = _core_geom(k)
    xs = np.zeros((C, XROWS, XW, B), dtype=x_chwb.dtype)
    for h in range(XROWS):
        if s0 == 0:
            xs[:, h, 0:18] = x_chwb[:, r0 + h, 0:18]
            xs[:, h, 18:36] = x_chwb[:, r0 + h, 14:32]
        else:
            xs[:, h, 0:18] = x_chwb[:, r0 + h, 14:32]
            if r0 + h + 1 < H:
                xs[:, h, 18:36] = x_chwb[:, r0 + h + 1, 0:18]
    return xs


def _build_xp(x_chwb, k):
    """-> [XROWS, 128, PAIRS*B] f16, partition g*64+c = channel c of col 2cp+g."""
    xs = _build_xs(x_chwb, k) * np.float32(1.0 / WSCALE)
    # [C, XROWS, PAIRS, 2, B] -> [2, C, XROWS, PAIRS, B]
    xg = xs.reshape(C, XROWS, PAIRS, 2, B).transpose(3, 0, 1, 2, 4)
    xp = xg.reshape(128, XROWS, PAIRS * B).transpose(1, 0, 2)
    return np.ascontiguousarray(xp).astype(np.float16)


def _abs_pos(k, t):
    p = PPC * k + t
    return p // COLS, p % COLS


def _build_wt(weight, k):
    """weight [ROWS,COLS,O,C,KH,KW] f32 -> per-core [128, NPAIR*PAIRB] E3M4.

    Per pair p (positions te=2p, to=2p+1), per-partition byte layout:
      [0,1280):    te pair chunks, (u,q)-major, each [128, O]; partition
                   g*64+c holds w[o, c, u, v = 2q + g + par(t)]
      [1280,1920): 5 shared lone blocks [128, O]: partitions 0:64 = te tap
                   (u, 4), 64:128 = to tap (u, 0)
      [1920,3200): to pair chunks
    (te's operands live in [0,1920) so the leading DMA of pair 0 can be
    split there for a faster pipeline start.)
    """
    import ml_dtypes

    ii, jj = zip(*[_abs_pos(k, t) for t in range(PPC)])
    wc = weight[list(ii), list(jj)]  # [PPC, O, C, KH, KW] f32
    # t parity == jj parity == w2 parity for every core (offsets are even)
    WT = np.zeros((2, C, NPAIR, PAIRB), np.float32)  # [g, c, p, col]
    for u in range(KH):
        for q in range(2):
            for g in range(2):
                for half in range(2):
                    v = 2 * q + g + half
                    blk = wc[half::2, :, :, u, v]  # [NPAIR, O, C]
                    col = half * 1920 + (2 * u + q) * O
                    WT[g, :, :, col:col + O] = blk.transpose(2, 0, 1)
        WT[0, :, :, 1280 + u * O:1280 + (u + 1) * O] = wc[0::2, :, :, u, 4].transpose(2, 0, 1)
        WT[1, :, :, 1280 + u * O:1280 + (u + 1) * O] = wc[1::2, :, :, u, 0].transpose(2, 0, 1)
    wt = np.ascontiguousarray(WT.reshape(128, NPAIR * PAIRB) * np.float32(WSCALE))
    return wt.astype(ml_dtypes.float8_e3m4)


def _chunks(t):
    """Position t -> (di, 10 pair descriptors, 5 lone descriptors).

    Descriptors are (kind, u, cp, woff, g).  The device emits all pair
    chunks of an output block, then the lone (K=64) chunks grouped by
    parity, so 128<->64 PE tile-config switches happen per block, not per
    position.
    """
    di, w2 = _pos_slot(t)
    par = w2 % 2
    half = t % 2
    cp0 = (w2 + par) // 2
    cpl = (w2 + 4) // 2 if par == 0 else (w2 - 1) // 2
    pairs = [("pair", u, cp0 + q, half * 1920 + (2 * u + q) * O, 0)
             for u in range(KH) for q in range(2)]
    lones = [("lone", u, cpl, 1280 + u * O, par) for u in range(KH)]
    return di, pairs, lones


def _emulate_core(xp, wt, xr=None):
    """Pure-numpy emulation of the device program (mirrors AP arithmetic)."""
    xpf = xp.astype(np.float32)                      # [8, 128, PAIRS*B]
    wtf = wt.astype(np.float32).reshape(128, NPAIR, PAIRB)
    out = np.zeros((128, PPC, B), np.float32)
    for t in range(PPC):
        p = t // 2
        di, pairs, lones = _chunks(t)
        acc = np.zeros((128, B), np.float32)
        for kind, u, cp, woff, g in pairs + lones:
            if kind == "pair":
                lhsT = wtf[:, p, woff:woff + O]          # [128, 128]
                rhs = xpf[di + u, :, cp * B:(cp + 1) * B]  # [128, 64]
            else:
                lhsT = wtf[g * 64:(g + 1) * 64, p, woff:woff + O]
                rhs = xpf[di + u, g * 64:(g + 1) * 64, cp * B:(cp + 1) * B]
            acc += lhsT.T @ rhs
        out[:, t, :] = acc
    return out  # [O, PPC, B]; scale already folded via x/32 * w*32


def _assemble(outs):
    """list of 8 per-core [128, PPC*B] -> [B, O, ROWS, COLS] f32."""
    full = np.concatenate(
        [np.asarray(o, np.float32).reshape(O, PPC, B) for o in outs], axis=1)
    return np.ascontiguousarray(full.transpose(2, 0, 1)).reshape(B, O, ROWS, COLS)


_PROG_CACHE = {}


def _build_program():
    if "nc" in _PROG_CACHE:
        return _PROG_CACHE["nc"]
    import concourse.bass as bass
    import concourse.tile as tile
    from concourse import bacc, mybir

    f8, f16, f32 = mybir.dt.float8e3, mybir.dt.float16, mybir.dt.float32
    nc = bacc.Bacc("TRN2", target_bir_lowering=False, debug=False, num_devices=NCORES)
    xp_d = nc.dram_tensor("xp", [XROWS, 128, PAIRS * B], f16, kind="ExternalInput")
    wt_d = nc.dram_tensor("wt", [128, NPAIR * PAIRB], f8, kind="ExternalInput")
    out_d = nc.dram_tensor("out", [128, PPC * B], f16, kind="ExternalOutput")

    with tile.TileContext(nc) as tc:
        with tc.tile_pool(name="xpool", bufs=1) as xpool, \
             tc.tile_pool(name="wpool", bufs=WLA) as wpool, \
             tc.tile_pool(name="opool", bufs=3) as opool, \
             tc.tile_pool(name="psum", bufs=8, space="PSUM") as ppool:
            xp, wt, outp = xp_d.ap(), wt_d.ap(), out_d.ap()
            XT = [xpool.tile([128, PAIRS * B], f16, name=f"x{h}", tag=f"x{h}")
                  for h in range(XROWS)]
            # Weight DMAs carry GROUPS[i] pairs each: big transfers amortize
            # the per-DMA SEQ/DGE overhead (~1.2us) so two HWDGE queues keep
            # the 16 DMA engines saturated; the leading groups stay small
            # (and pair 0 is split) for a fast pipeline start.
            GROUPS = [1, 1] + [2] * 23 + [1]
            g0 = [0]
            for n in GROUPS:
                g0.append(g0[-1] + n)
            pair_loc = {}
            for gi, n in enumerate(GROUPS):
                for l in range(n):
                    pair_loc[g0[gi] + l] = (gi, l * PAIRB)
            wtiles = [wpool.tile([128, n * PAIRB], f8, name=f"w{gi}", tag="wt")
                      for gi, n in enumerate(GROUPS)]
            weng = [nc.scalar, nc.sync]

            def load_x(h, hf, eng):
                eng.dma_start(XT[h][:, hf * XHALF:(hf + 1) * XHALF],
                              xp[h, :, hf * XHALF:(hf + 1) * XHALF])

            def load_w(gi):
                c0, c1 = g0[gi] * PAIRB, g0[gi + 1] * PAIRB
                weng[gi % 2].dma_start(wtiles[gi][:], wt[:, c0:c1])

            # Emission order == per-queue FIFO order.  Position 0 needs x
            # rows 0-4 (first halves) + pair 0's [0,1920) slice; x rows 5-7
            # only matter from t=28, second halves (xb) from t=14.
            nc.sync.dma_start(wtiles[0][:, 0:1920], wt[:, 0:1920])
            for h in range(3):
                load_x(h, 0, nc.scalar)
            nc.sync.dma_start(wtiles[0][:, 1920:PAIRB], wt[:, 1920:PAIRB])
            load_x(3, 0, nc.scalar)
            load_x(4, 0, nc.scalar)
            load_w(1)   # sync
            load_w(2)   # scalar
            load_w(3)   # sync
            for h in range(5, XROWS):
                load_x(h, 0, nc.scalar)
            load_w(4)
            for h in range(0, XROWS, 2):
                load_x(h, 1, nc.sync)
            for h in range(1, XROWS, 2):
                load_x(h, 1, nc.scalar)
            for gi in range(5, len(GROUPS)):
                load_w(gi)  # flow-controlled by wpool depth

            def mm(ps, p, di, ch, start, stop):
                kind, u, cp, woff, g = ch
                gi, poff = pair_loc[p]
                if kind == "pair":
                    lhsT = wtiles[gi][:, poff + woff:poff + woff + O]
                    rhs = XT[di + u][:, cp * B:(cp + 1) * B]
                else:
                    lhsT = wtiles[gi][g * 64:g * 64 + 64, poff + woff:poff + woff + O]
                    rhs = XT[di + u][g * 64:g * 64 + 64, cp * B:(cp + 1) * B]
                nc.tensor.matmul(ps[:], lhsT, rhs, start=start, stop=stop)

            for t0 in range(0, PPC, OBLK):
                otile = opool.tile([128, OBLK * B], f16, tag="ot")
                pss, parts = {}, {}
                for t in range(t0, t0 + OBLK):
                    di, pairs, lones = _chunks(t)
                    ps = ppool.tile([128, B], f32, tag="ps")
                    pss[t], parts[t] = ps, (di, lones)
                    for i, ch in enumerate(pairs):
                        mm(ps, t // 2, di, ch, start=(i == 0), stop=False)
                for par in (t0 % 2, 1 - t0 % 2):
                    for t in range(t0, t0 + OBLK):
                        if t % 2 != par:
                            continue
                        di, lones = parts[t]
                        for i, ch in enumerate(lones):
                            mm(pss[t], t // 2, di, ch, start=False, stop=(i == KH - 1))
                        nc.vector.tensor_copy(
                            otile[:, (t - t0) * B:(t - t0 + 1) * B], pss[t][:])
                oeng = nc.sync if t0 + OBLK >= PPC else nc.gpsimd
                oeng.dma_start(outp[:, t0 * B:(t0 + OBLK) * B], otile[:])

    nc.compile()
    _PROG_CACHE["nc"] = nc
    return nc


def _make_in_maps(x, weight):
    x_chwb = np.ascontiguousarray(
        np.asarray(x, np.float32).transpose(1, 2, 3, 0))
    w32 = np.asarray(weight, np.float32)
    return [{"xp": _build_xp(x_chwb, k), "wt": _build_wt(w32, k)}
            for k in range(NCORES)]


def kernel(x, weight):
    from concourse.bass_utils import run_bass_kernel_spmd

    nc = _build_program()
    in_maps = _make_in_maps(x, weight)
    res = run_bass_kernel_spmd(nc, in_maps, core_ids=list(range(NCORES)))
    return _assemble([res.results[k]["out"] for k in range(NCORES)])


# revision 47
# speedup vs baseline: 1.1612x; 1.0015x over previous
"""Locally-connected convolution (unshared weights) on 8 Trainium2 NeuronCores.

out[b,o,i,j] = sum_{c,u,v} x[b,c,i+u,j+v] * weight[i,j,o,c,u,v]
  x: [64, 64, 32, 32] f32, weight: [28, 28, 128, 64, 5, 5] f32 -> out [64, 128, 28, 28]

Strategy (v2, "pairs" layout): each of the 784 output positions is an
independent GEMM [K=1600] x [K=1600, O=128] over B=64 batch vectors.  Shard
the 784 positions across 8 cores (98 each, raster-contiguous).

Weights are cast host-side to float8 E3M4 (x32 scale; x is pre-scaled by
1/32 in fp16 so no on-device rescale is needed).  This halves the dominant
HBM traffic (weights are used exactly once).  The matmul is "flipped"
vs. v1: weights are the stationary lhsT [K, O=128] and x is the moving rhs
[K, B=64], so each matmul streams only 64 rows and fills all 128 PSUM
partitions -> 2x fewer PE cycles.

K decomposition per position (1600 = 25 taps x 64 ch):
  x is stored ONCE per core in "column pair" layout: partitions 0-63 hold
  channels of EVEN input columns, partitions 64-127 channels of ODD columns;
  free axis is (row h, column-pair cp, batch b).  A [128, 64] slice at
  (h, cp) yields two adjacent-column taps at K=128.  Positions alternate
  column parity, so every position gets 10 such pair chunks (u x 2) plus 5
  leftover single taps (K=64, lower partitions for even positions / upper
  for odd).  Leftover-tap weights of an (even, odd) position pair share one
  [128, 128] block -> zero padding in the weight stream (20.07 MB/core).
"""

import numpy as np

B, C, H, W = 64, 64, 32, 32
ROWS = COLS = 28
O, KH, KW = 128, 5, 5
NCORES = 8
PPC = (ROWS * COLS) // NCORES  # 98 positions per core
NPAIR = PPC // 2               # 49 (even, odd) position pairs
XROWS, XW = 8, 36              # sheared x grid: 8 input rows x 36 cols
PAIRS = XW // 2                # 18 column pairs per sheared row
PAIRB = 2 * 10 * O + 5 * O     # weight bytes per partition per pair = 3200
WSCALE = 32.0                  # weights x32 into E3M4 range; x carries /32
OBLK = 7                       # positions per output block/DMA
WLA = 16                       # wtile pool depth (pairs in flight)
XHALF = 9 * B                  # x row loaded in two 9-pair halves
XRROWS = 6                     # row-pair workspace rows (di+u0 <= 5)


def _core_geom(k):
    p0 = PPC * k
    return p0 // COLS, p0 % COLS  # r0 (first input/output row), s0 in {0, 14}


def _pos_slot(t):
    """Relative position t in [0,98) -> (di, w2) grid coords shared by all cores."""
    di, jj = t // COLS, t % COLS
    return di, jj + (4 if jj >= 14 else 0)


def _build_xs(x_chwb, k):
    """x_chwb: [C,H,W,B] f32 -> sheared per-core grid [C, XROWS, XW, B]."""
    r0, s0 = _core_geom(k)
    xs = np.zeros((C, XROWS, XW, B), dtype=x_chwb.dtype)
    for h in range(XROWS):
        if s0 == 0:
            xs[:, h, 0:18] = x_chwb[:, r0 + h, 0:18]
            xs[:, h, 18:36] = x_chwb[:, r0 + h, 14:32]
        else:
            xs[:, h, 0:18] = x_chwb[:, r0 + h, 14:32]
            if r0 + h + 1 < H:
                xs[:, h, 18:36] = x_chwb[:, r0 + h + 1, 0:18]
    return xs


def _build_xp(x_chwb, k):
    """-> [XROWS, 128, PAIRS*B] f16, partition g*64+c = channel c of col 2cp+g."""
    xs = _build_xs(x_chwb, k) * np.float32(1.0 / WSCALE)
    # [C, XROWS, PAIRS, 2, B] -> [2, C, XROWS, PAIRS, B]
    xg = xs.reshape(C, XROWS, PAIRS, 2, B).transpose(3, 0, 1, 2, 4)
    xp = xg.reshape(128, XROWS, PAIRS * B).transpose(1, 0, 2)
    return np.ascontiguousarray(xp).astype(np.float16)


def _abs_pos(k, t):
    p = PPC * k + t
    return p // COLS, p % COLS


def _build_wt(weight, k):
    """weight [ROWS,COLS,O,C,KH,KW] f32 -> per-core [128, NPAIR*PAIRB] E3M4.

    Per pair p (positions te=2p, to=2p+1), per-partition byte layout:
      [0,1280):    te pair chunks, (u,q)-major, each [128, O]; partition
                   g*64+c holds w[o, c, u, v = 2q + g + par(t)]
      [1280,1920): 5 shared lone blocks [128, O]: partitions 0:64 = te tap
                   (u, 4), 64:128 = to tap (u, 0)
      [1920,3200): to pair chunks
    (te's operands live in [0,1920) so the leading DMA of pair 0 can be
    split there for a faster pipeline start.)
    """
    import ml_dtypes

    ii, jj = zip(*[_abs_pos(k, t) for t in range(PPC)])
    wc = weight[list(ii), list(jj)]  # [PPC, O, C, KH, KW] f32
    # t parity == jj parity == w2 parity for every core (offsets are even)
    WT = np.zeros((2, C, NPAIR, PAIRB), np.float32)  # [g, c, p, col]
    for u in range(KH):
        for q in range(2):
            for g in range(2):
                for half in range(2):
                    v = 2 * q + g + half
                    blk = wc[half::2, :, :, u, v]  # [NPAIR, O, C]
                    col = half * 1920 + (2 * u + q) * O
                    WT[g, :, :, col:col + O] = blk.transpose(2, 0, 1)
        WT[0, :, :, 1280 + u * O:1280 + (u + 1) * O] = wc[0::2, :, :, u, 4].transpose(2, 0, 1)
        WT[1, :, :, 1280 + u * O:1280 + (u + 1) * O] = wc[1::2, :, :, u, 0].transpose(2, 0, 1)
    wt = np.ascontiguousarray(WT.reshape(128, NPAIR * PAIRB) * np.float32(WSCALE))
    return wt.astype(ml_dtypes.float8_e3m4)


def _chunks(t):
    """Position t -> (di, 10 pair descriptors, 5 lone descriptors).

    Descriptors are (kind, u, cp, woff, g).  The device emits all pair
    chunks of an output block, then the lone (K=64) chunks grouped by
    parity, so 128<->64 PE tile-config switches happen per block, not per
    position.
    """
    di, w2 = _pos_slot(t)
    par = w2 % 2
    half = t % 2
    cp0 = (w2 + par) // 2
    cpl = (w2 + 4) // 2 if par == 0 else (w2 - 1) // 2
    pairs = [("pair", u, cp0 + q, half * 1920 + (2 * u + q) * O, 0)
             for u in range(KH) for q in range(2)]
    lones = [("lone", u, cpl, 1280 + u * O, par) for u in range(KH)]
    return di, pairs, lones


def _emulate_core(xp, wt, xr=None):
    """Pure-numpy emulation of the device program (mirrors AP arithmetic)."""
    xpf = xp.astype(np.float32)                      # [8, 128, PAIRS*B]
    wtf = wt.astype(np.float32).reshape(128, NPAIR, PAIRB)
    out = np.zeros((128, PPC, B), np.float32)
    for t in range(PPC):
        p = t // 2
        di, pairs, lones = _chunks(t)
        acc = np.zeros((128, B), np.float32)
        for kind, u, cp, woff, g in pairs + lones:
            if kind == "pair":
                lhsT = wtf[:, p, woff:woff + O]          # [128, 128]
                rhs = xpf[di + u, :, cp * B:(cp + 1) * B]  # [128, 64]
            else:
                lhsT = wtf[g * 64:(g + 1) * 64, p, woff:woff + O]
                rhs = xpf[di + u, g * 64:(g + 1) * 64, cp * B:(cp + 1) * B]
            acc += lhsT.T @ rhs
        out[:, t, :] = acc
    return out  # [O, PPC, B]; scale already folded via x/32 * w*32


def _assemble(outs):
    """list of 8 per-core [128, PPC*B] -> [B, O, ROWS, COLS] f32."""
    full = np.concatenate(
        [np.asarray(o, np.float32).reshape(O, PPC, B) for o in outs], axis=1)
    return np.ascontiguousarray(full.transpose(2, 0, 1)).reshape(B, O, ROWS, COLS)


_PROG_CACHE = {}


def _build_program():
    if "nc" in _PROG_CACHE:
        return _PROG_CACHE["nc"]
    import concourse.bass as bass
    import concourse.tile as tile
    from concourse import bacc, mybir

    f8, f16, f32 = mybir.dt.float8e3, mybir.dt.float16, mybir.dt.float32
    nc = bacc.Bacc("TRN2", target_bir_lowering=False, debug=False, num_devices=NCORES)
    xp_d = nc.dram_tensor("xp", [XROWS, 128, PAIRS * B], f16, kind="ExternalInput")
    wt_d = nc.dram_tensor("wt", [128, NPAIR * PAIRB], f8, kind="ExternalInput")
    out_d = nc.dram_tensor("out", [128, PPC * B], f16, kind="ExternalOutput")

    with tile.TileContext(nc) as tc:
        with tc.tile_pool(name="xpool", bufs=1) as xpool, \
             tc.tile_pool(name="wpool", bufs=WLA) as wpool, \
             tc.tile_pool(name="opool", bufs=3) as opool, \
             tc.tile_pool(name="psum", bufs=8, space="PSUM") as ppool:
            xp, wt, outp = xp_d.ap(), wt_d.ap(), out_d.ap()
            XT = [xpool.tile([128, PAIRS * B], f16, name=f"x{h}", tag=f"x{h}")
                  for h in range(XROWS)]
            # Weight DMAs carry GROUPS[i] pairs each: big transfers amortize
            # the per-DMA SEQ/DGE overhead (~1.2us) so two HWDGE queues keep
            # the 16 DMA engines saturated; the leading groups stay small
            # (and pair 0 is split) for a fast pipeline start.
            GROUPS = [1, 1] + [2] * 23 + [1]
            g0 = [0]
            for n in GROUPS:
                g0.append(g0[-1] + n)
            pair_loc = {}
            for gi, n in enumerate(GROUPS):
                for l in range(n):
                    pair_loc[g0[gi] + l] = (gi, l * PAIRB)
            wtiles = [wpool.tile([128, n * PAIRB], f8, name=f"w{gi}", tag="wt")
                      for gi, n in enumerate(GROUPS)]
            weng = [nc.scalar, nc.sync]

            def load_x(h, hf, eng):
                eng.dma_start(XT[h][:, hf * XHALF:(hf + 1) * XHALF],
                              xp[h, :, hf * XHALF:(hf + 1) * XHALF])

            def load_w(gi):
                c0, c1 = g0[gi] * PAIRB, g0[gi + 1] * PAIRB
                weng[gi % 2].dma_start(wtiles[gi][:], wt[:, c0:c1])

            # Emission order == per-queue FIFO order.  Position 0 needs x
            # rows 0-4 (first halves) + pair 0's [0,1920) slice; x rows 5-7
            # only matter from t=28, second halves (xb) from t=14.
            nc.sync.dma_start(wtiles[0][:, 0:1920], wt[:, 0:1920])
            for h in range(3):
                load_x(h, 0, nc.scalar)
            nc.sync.dma_start(wtiles[0][:, 1920:PAIRB], wt[:, 1920:PAIRB])
            load_x(3, 0, nc.scalar)
            load_x(4, 0, nc.scalar)
            load_w(1)   # sync
            load_w(2)   # scalar
            load_w(3)   # sync
            for h in range(5, XROWS):
                load_x(h, 0, nc.scalar)
            load_w(4)
            for h in range(0, XROWS, 2):
                load_x(h, 1, nc.sync)
            for h in range(1, XROWS, 2):
                load_x(h, 1, nc.scalar)
            for gi in range(5, len(GROUPS)):
                load_w(gi)  # flow-controlled by wpool depth

            def mm(ps, p, di, ch, start, stop):
                kind, u, cp, woff, g = ch
                gi, poff = pair_loc[p]
                if kind == "pair":
                    lhsT = wtiles[gi][:, poff + woff:poff + woff + O]
                    rhs = XT[di + u][:, cp * B:(cp + 1) * B]
                else:
                    lhsT = wtiles[gi][g * 64:g * 64 + 64, poff + woff:poff + woff + O]
                    rhs = XT[di + u][g * 64:g * 64 + 64, cp * B:(cp + 1) * B]
                nc.tensor.matmul(ps[:], lhsT, rhs, start=start, stop=stop)

            for t0 in range(0, PPC, OBLK):
                otile = opool.tile([128, OBLK * B], f16, tag="ot")
                pss, parts = {}, {}
                for t in range(t0, t0 + OBLK):
                    di, pairs, lones = _chunks(t)
                    ps = ppool.tile([128, B], f32, tag="ps")
                    pss[t], parts[t] = ps, (di, lones)
                    for i, ch in enumerate(pairs):
                        mm(ps, t // 2, di, ch, start=(i == 0), stop=False)
                for par in (t0 % 2, 1 - t0 % 2):
                    for t in range(t0, t0 + OBLK):
                        if t % 2 != par:
                            continue
                        di, lones = parts[t]
                        for i, ch in enumerate(lones):
                            mm(pss[t], t // 2, di, ch, start=False, stop=(i == KH - 1))
                        nc.vector.tensor_copy(
                            otile[:, (t - t0) * B:(t - t0 + 1) * B], pss[t][:])
                oeng = nc.sync if t0 + OBLK >= PPC else nc.gpsimd
                oeng.dma_start(outp[:, t0 * B:(t0 + OBLK) * B], otile[:])

    nc.compile()
    _PROG_CACHE["nc"] = nc
    return nc


def _make_in_maps(x, weight):
    x_chwb = np.ascontiguousarray(
        np.asarray(x, np.float32).transpose(1, 2, 3, 0))
    w32 = np.asarray(weight, np.float32)
    return [{"xp": _build_xp(x_chwb, k), "wt": _build_wt(w32, k)}
            for k in range(NCORES)]


def kernel(x, weight):
    from concourse.bass_utils import run_bass_kernel_spmd

    nc = _build_program()
    in_maps = _make_in_maps(x, weight)
    res = run_bass_kernel_spmd(nc, in_maps, core_ids=list(range(NCORES)))
    return _assemble([res.results[k]["out"] for k in range(NCORES)])
